# revision 63
# baseline (speedup 1.0000x reference)
"""HAN metapath-attention kernel for 8 Trainium2 NeuronCores (Bass/Tile).

Reference computation (B=512, P=64, K=8, D=512, T=50000):
    ref_embs = inputs[nbr_batch, nbr_job]            # [B,P,K,D] gather
    raw_s    = einsum('bpd,bpkd->bpk', inputs, ref_embs)
    sim      = softmax(where(mask, raw_s, -1e9)) * mask
    out      = concat([inputs, einsum('bpk,bpkt->bpt', sim, title[nbr_title])], -1)
    returns [B*P, 2D] f32

Sharding: data-parallel over flat rows r = b*P + p; core c owns rows
[c*4096, (c+1)*4096). The flattened `inputs` gather table is replicated to
every core's HBM; `title_emb_mat` is compacted per core (only referenced
rows; always <= 32768 distinct so local ids fit int16). No cross-core
traffic is needed.

Sparsity: mask ~ Bernoulli(1/2) makes half the neighbor slots contribute
EXACTLY zero (masked slots' exp underflows to exact 0.0 in the softmax),
so the kernel only gathers valid slots. Focals are sorted by valid-count v
(descending); each 128-focal tile gathers W = max v in the tile slots per
focal, with a 0/-1e9 logit bias covering the v < W remainder. v=0 focals
(graph half exactly 0) never touch the device. Exact, not approximate.

Random-row dma_gather on TRN2 is descriptor-latency-bound, not
bandwidth-bound, so everything minimizes descriptor count and keeps the
SWDGE rings busy: all gathered data travels bf16 (tables, focal rows,
output; ~5e-3 output rel err vs the 2e-2 budget), consecutive tiles group
into <=1024-index superblock gathers (more overflows the descriptor ring
and wedges the device), J gathers rotate over 3 SWDGE queues with T on a
4th (ring depth gates the Pool engine's issue rate), and per-op compute is
two-phased (dots/logits/exp for all tiles, then softmax/weighted-sum) so
the in-order DVE never stalls on the Activation engine's exp. The weighted
title sum accumulates in bf16 (all-16-bit STT operands run ~2x on DVE).
W==1 tiles skip the J gather and softmax entirely (sim == 1 exactly) and
store the gathered title row directly. The focal half of the output is
host-assembled (it is literally the input rows); device output rows are
unpermuted on the host.
"""

import sys
import time

if "/opt/trn_rl_repo" not in sys.path:
    sys.path.insert(0, "/opt/trn_rl_repo")

import numpy as np

import concourse.bacc as bacc
import concourse.bass as bass
import concourse.tile as tile
from concourse import mybir

B, P, K, D, T = 512, 64, 8, 512, 50000
NCORES = 8
R = B * P // NCORES  # 4096 focal rows per core
F32 = mybir.dt.float32
BF16 = mybir.dt.bfloat16
I16 = mybir.dt.int16
SLOTCAP = 8  # max gathered slots (summed tile widths) per superblock op;
# 128*SLOTCAP = 1024 indices per dma_gather (larger overflows the SWDGE
# descriptor ring)

import ml_dtypes

NP_BF16 = ml_dtypes.bfloat16


# ------------------------------------------------------------------ layout --


def _layout(widths):
    """Emission-order op layout shared by builder and host prep.

    `widths` is the per-tile slot width vector (descending); tile t covers
    focal rows [128t, 128t+128) in the bucket-sorted order, gathering
    widths[t] neighbor slots per focal. Consecutive tiles are grouped into
    superblock ops of at most SLOTCAP summed width: one T gather over all
    slots, one J gather over slots of W>=2 tiles only (a W==1 tile's sim is
    exactly its mask, so neighbor-job rows and the whole softmax are
    skipped), one sequential focal load, per-tile compute + stores.

    Each op: dict(tiles=[(W, so_t, so_j, mc)], nt, ns, nsj, jcol, tcol,
    mcol0, rowbase). Returns (ops, idx_cols, mask_cols, rows).
    """
    ops = []
    col = 0
    mcol = 0
    rowbase = 0
    t = 0
    ntiles = len(widths)
    while t < ntiles:
        tiles = []
        ns = 0
        nsj = 0
        while t < ntiles and len(tiles) < SLOTCAP and ns + widths[t] <= SLOTCAP:
            W = widths[t]
            tiles.append((W, ns, nsj if W >= 2 else -1, mcol))
            ns += W
            if W >= 2:
                nsj += W
            mcol += W
            t += 1
        op = dict(
            tiles=tiles,
            nt=len(tiles),
            ns=ns,
            nsj=nsj,
            rowbase=rowbase,
            jcol=col,
        )
        col += (128 * nsj) // 16
        op["tcol"] = col
        col += (128 * ns) // 16
        ops.append(op)
        rowbase += len(tiles) * 128
    return ops, col, mcol, rowbase


# ----------------------------------------------------------------- builder --


def _build_program(plan, niter=1):
    """plan = (per-tile width tuple, u_pad). niter>1 wraps the pass in a
    For_i loop (bench-only; makes device time dominate one execution)."""
    widths = list(plan[0])
    u_pad = plan[1]
    ops, idx_cols, mask_cols, total_rows = _layout(widths)
    assert total_rows > 0

    nc = bacc.Bacc(
        "TRN2", target_bir_lowering=False, debug=False, num_swdge_queues=4
    )
    # gathered tables / focal rows / output all travel as bf16: halves the
    # random-row gather traffic that bounds this kernel. Softmax numerics
    # stay in f32; bf16 operand error contributes ~3e-3 output rel err.
    emb = nc.dram_tensor("emb", [B * P, D], BF16, kind="ExternalInput")
    title = nc.dram_tensor("title", [u_pad, D], BF16, kind="ExternalInput")
    # bucket-ordered focal rows (host pre-permuted): row rb + t*128 + p is the
    # focal embedding of (tile t, partition p) -> sequential HWDGE loads
    focal = nc.dram_tensor("focal", [total_rows, D], BF16, kind="ExternalInput")
    gidx16 = nc.dram_tensor("gidx16", [128, idx_cols], I16, kind="ExternalInput")
    maskf = nc.dram_tensor("maskf", [128, mask_cols], F32, kind="ExternalInput")
    maskb = nc.dram_tensor("maskb", [128, mask_cols], F32, kind="ExternalInput")
    outg = nc.dram_tensor("outg", [total_rows, D], BF16, kind="ExternalOutput")

    with tile.TileContext(nc) as tc:
        with (
            tc.tile_pool(name="idxp", bufs=1) as idxp,
            tc.tile_pool(name="fp", bufs=3) as fp,
            tc.tile_pool(name="jp", bufs=3) as jp,
            tc.tile_pool(name="tp", bufs=3) as tp,
            tc.tile_pool(name="wp", bufs=4) as wp,
            tc.tile_pool(name="sp", bufs=6) as sp,
        ):
            gx = idxp.tile([128, idx_cols], I16)
            mf = idxp.tile([128, mask_cols], F32)
            mb = idxp.tile([128, mask_cols], F32)
            nc.sync.dma_start(out=gx[:], in_=gidx16[:])
            nc.sync.dma_start(out=mf[:], in_=maskf[:])
            nc.sync.dma_start(out=mb[:], in_=maskb[:])

            import contextlib

            loop_ctx = (
                tc.For_i(0, niter, 1) if niter > 1 else contextlib.nullcontext()
            )
            with loop_ctx:
                for opi, op in enumerate(ops):
                    nt, ns, nsj = op["nt"], op["ns"], op["nsj"]
                    rb0 = op["rowbase"]
                    Fs = fp.tile([128, SLOTCAP, D], BF16, tag="F")
                    nc.sync.dma_start(
                        out=Fs[:, :nt, :],
                        in_=focal[rb0 : rb0 + nt * 128, :].rearrange(
                            "(t p) d -> p t d", p=128
                        ),
                    )
                    if nsj > 0:
                        jn = 128 * nsj
                        Js = jp.tile([128, SLOTCAP, D], BF16, tag="J")
                        nc.gpsimd.dma_gather(
                            Js[:, :nsj, :],
                            emb[:],
                            gx[:, op["jcol"] : op["jcol"] + jn // 16],
                            jn,
                            jn,
                            D,
                            # J and T interleave across all 4 rings: ring
                            # depth gates the Pool engine's issue rate (a
                            # chain on one ring advances at transfer speed)
                            queue_num=(2 * opi) % 4,
                        )
                    tn = 128 * ns
                    Ts = tp.tile([128, SLOTCAP, D], BF16, tag="T")
                    nc.gpsimd.dma_gather(
                        Ts[:, :ns, :],
                        title[:],
                        gx[:, op["tcol"] : op["tcol"] + tn // 16],
                        tn,
                        tn,
                        D,
                        queue_num=(2 * opi + 1) % 4,
                    )
                    # phase A (per tile): dots -> logits -> -max -> exp.
                    # phase B (per tile): 1/sum -> sim -> weighted title sum.
                    # Splitting lets tile t+1's dots fill DVE while tile t's
                    # exp runs on the Activation engine (DVE is in-order, so
                    # an inline exp would stall it every tile).
                    es = {}
                    for t, (W, so, soj, mc) in enumerate(op["tiles"]):
                        if W == 1:
                            continue
                        # dots[:, k] = sum_d F * J_k (fused product+row-reduce)
                        dots = sp.tile([128, 8], F32, tag="dots")
                        prod = wp.tile([128, D], BF16, tag="prod")
                        for k in range(W):
                            nc.vector.scalar_tensor_tensor(
                                out=prod[:],
                                in0=Fs[:, t, :],
                                scalar=1.0,
                                in1=Js[:, soj + k, :],
                                op0=mybir.AluOpType.mult,
                                op1=mybir.AluOpType.mult,
                                accum_out=dots[:, k : k + 1],
                            )
                        # masked logits = dots + (0 | -1e9)
                        logits = sp.tile([128, 8], F32, tag="logits")
                        nc.vector.tensor_tensor(
                            out=logits[:, :W],
                            in0=dots[:, :W],
                            in1=mb[:, mc : mc + W],
                            op=mybir.AluOpType.add,
                        )
                        negM = sp.tile([128, 1], F32, tag="negM")
                        nc.vector.tensor_reduce(
                            out=negM[:],
                            in_=logits[:, :W],
                            axis=mybir.AxisListType.X,
                            op=mybir.AluOpType.max,
                            negate=True,
                        )
                        e = sp.tile([128, 8], F32, tag="e")
                        nc.scalar.activation(
                            out=e[:, :W],
                            in_=logits[:, :W],
                            func=mybir.ActivationFunctionType.Exp,
                            bias=negM[:, 0:1],
                            scale=1.0,
                        )
                        es[t] = e
                    for t, (W, so, soj, mc) in enumerate(op["tiles"]):
                        rb = rb0 + t * 128
                        if W == 1:
                            # every packed focal here has v==1 -> sim == 1
                            # exactly; dummy rows are host-discarded. Store
                            # the gathered title row directly.
                            nc.sync.dma_start(
                                out=outg[rb : rb + 128, :], in_=Ts[:, so, :]
                            )
                            continue
                        e = es[t]
                        ssum = sp.tile([128, 1], F32, tag="ssum")
                        nc.vector.tensor_reduce(
                            out=ssum[:],
                            in_=e[:, :W],
                            axis=mybir.AxisListType.X,
                            op=mybir.AluOpType.add,
                        )
                        rr = sp.tile([128, 1], F32, tag="rr")
                        nc.vector.reciprocal(out=rr[:], in_=ssum[:])
                        # sim = e/sum (masked slots' e underflow to exact 0,
                        # so no mask multiply is needed; all-masked rows are
                        # host-discarded)
                        sim = sp.tile([128, 8], F32, tag="sim")
                        nc.vector.scalar_tensor_tensor(
                            out=sim[:, :W],
                            in0=e[:, :W],
                            scalar=rr[:, 0:1],
                            in1=e[:, :W],
                            op0=mybir.AluOpType.mult,
                            op1=mybir.AluOpType.bypass,
                        )
                        # weighted title sum (fused mult+add chain, bf16
                        # accumulator: all-16-bit operands halve DVE port
                        # traffic; adds ~1e-3 rel err, well within budget);
                        # the last op writes the bf16 store tile directly
                        acc = wp.tile([128, D], BF16, tag="acc")
                        accb = wp.tile([128, D], BF16, tag="accb")
                        nc.vector.scalar_tensor_tensor(
                            out=acc[:],
                            in0=Ts[:, so, :],
                            scalar=sim[:, 0:1],
                            in1=Ts[:, so, :],
                            op0=mybir.AluOpType.mult,
                            op1=mybir.AluOpType.bypass,
                        )
                        for k in range(1, W):
                            nc.vector.scalar_tensor_tensor(
                                out=(accb[:] if k == W - 1 else acc[:]),
                                in0=Ts[:, so + k, :],
                                scalar=sim[:, k : k + 1],
                                in1=acc[:],
                                op0=mybir.AluOpType.mult,
                                op1=mybir.AluOpType.add,
                            )
                        nc.sync.dma_start(out=outg[rb : rb + 128, :], in_=accb[:])
    nc.finalize()
    return nc


# --------------------------------------------------------------- host prep --


def _wrap_ops(flat_lists):
    """Concat per-op flat index lists into the [128, cols] int16 idx tile.
    dma_gather consumes list l with out[p, j, :] = tbl[l[j*128+p]]; element i
    of each op's list lives at [i % 16, i // 16] of its column block,
    replicated x8 across partition groups (one per Q7 core)."""
    blocks = []
    for L in flat_lists:
        L = np.asarray(L, dtype=np.int16)
        assert len(L) % 16 == 0
        blocks.append(L.reshape(-1, 16).T)
    w = np.concatenate(blocks, axis=1)
    return np.ascontiguousarray(np.tile(w, (8, 1)))


def _sparse_host(inputs, title_emb_mat, nbr_batch, nbr_job, nbr_title, nbr_mask):
    """Bucket/compact per core. Returns None if no valid slots exist anywhere
    (output is then pure host assembly), else
    (plan, in_maps, row_focal per core, emb)."""
    inputs = np.asarray(inputs, dtype=np.float32)
    title_emb_mat = np.asarray(title_emb_mat, dtype=np.float32)
    emb = np.ascontiguousarray(inputs.reshape(B * P, D))
    emb16 = emb.astype(NP_BF16)
    title16 = title_emb_mat.astype(NP_BF16)
    jidx = (
        np.asarray(nbr_batch, dtype=np.int64) * P + np.asarray(nbr_job, dtype=np.int64)
    ).reshape(B * P, K)
    tidx = np.asarray(nbr_title, dtype=np.int64).reshape(B * P, K)
    m = np.asarray(nbr_mask, dtype=np.int64).reshape(B * P, K)

    percore = []
    for c in range(NCORES):
        rows = slice(c * R, (c + 1) * R)
        mrow = m[rows]
        v = mrow.sum(1)
        # valid slots first, ascending k among valid (keeps the fp reduce
        # order equal to the reference: adding exact zeros is an fp no-op)
        order = np.argsort(-mrow, axis=1, kind="stable")
        js = np.take_along_axis(jidx[rows], order, 1)
        tits = np.take_along_axis(tidx[rows], order, 1)
        valid_t = tits[mrow.astype(bool)[np.arange(R)[:, None], order]]
        uniq = np.unique(valid_t)
        lut = np.zeros(T, dtype=np.int64)
        lut[uniq] = np.arange(len(uniq))
        # focal processing order: valid-count descending (stable)
        sel = np.argsort(-v, kind="stable")
        sel = sel[v[sel] > 0]
        percore.append(dict(v=v, js=js, tl=lut[tits], uniq=uniq, sel=sel))

    # common per-tile width plan: elementwise max over cores of each core's
    # sorted valid-count profile at tile boundaries
    ntiles = max(-(-len(pc["sel"]) // 128) for pc in percore)
    if ntiles == 0:
        return None
    widths = np.zeros(ntiles, dtype=np.int64)
    for pc in percore:
        vs = pc["v"][pc["sel"]]  # descending
        for t in range(ntiles):
            if t * 128 < len(vs):
                widths[t] = max(widths[t], vs[t * 128])
    widths = np.maximum(widths, 1)
    u_pad = max(512, -(-max(len(pc["uniq"]) for pc in percore) // 512) * 512)
    ops, idx_cols, mask_cols, total_rows = _layout(tuple(widths))

    in_maps = []
    row_focal_all = []
    for c in range(NCORES):
        pc = percore[c]
        v, js, tl, selc = pc["v"], pc["js"], pc["tl"], pc["sel"]
        sel_pad = np.full(total_rows, -1, dtype=np.int64)
        sel_pad[: len(selc)] = selc
        flat_lists = []
        mfs = np.zeros((128, mask_cols), dtype=np.float32)
        mbs = np.full((128, mask_cols), -1e9, dtype=np.float32)
        focal_perm = np.zeros(total_rows, dtype=np.int64)
        for op in ops:
            jparts, tparts = [], []
            for t, (W, so, soj, mc) in enumerate(op["tiles"]):
                r0 = op["rowbase"] + t * 128
                sel = sel_pad[r0 : r0 + 128]
                focs = np.where(sel < 0, 0, sel)  # dummy focal -> row 0
                vv = np.where(sel < 0, 0, v[focs])  # dummy -> fully masked
                focal_perm[r0 : r0 + 128] = focs
                valid = np.arange(W)[None, :] < vv[:, None]  # [128, W]
                if soj >= 0:
                    jparts.append(np.where(valid, js[focs, :W], 0).T.reshape(-1))
                tparts.append(np.where(valid, tl[focs, :W], 0).T.reshape(-1))
                mfs[:, mc : mc + W] = valid.astype(np.float32)
                mbs[:, mc : mc + W] = (valid.astype(np.float32) - 1.0) * 1e9
            if jparts:
                flat_lists.append(np.concatenate(jparts))
            flat_lists.append(np.concatenate(tparts))
        row_focal_all.append(sel_pad)

        tloc = np.zeros((u_pad, D), dtype=NP_BF16)
        tloc[: len(pc["uniq"])] = title16[pc["uniq"]]
        in_maps.append(
            {
                "emb": emb16,
                "title": tloc,
                "focal": np.ascontiguousarray(emb16[c * R + focal_perm]),
                "gidx16": _wrap_ops(flat_lists),
                "maskf": mfs,
                "maskb": mbs,
            }
        )
    plan = (tuple(int(w) for w in widths), u_pad)
    return plan, in_maps, row_focal_all, emb


# ------------------------------------------------------------------ runner --

_RUNNERS = {}


class _Runner:
    """Caches the sharded jit executable for one program variant so repeated
    executions skip retracing/recompiling (adapted from
    concourse.bass2jax.run_bass_via_pjrt's multi-core branch)."""

    def __init__(self, plan, niter):
        import jax
        from jax.experimental.shard_map import shard_map
        from jax.sharding import Mesh, NamedSharding, PartitionSpec

        from concourse import mybir as _mb
        from concourse.bass2jax import (
            _bass_exec_p,
            install_neuronx_cc_hook,
            partition_id_tensor,
        )

        install_neuronx_cc_hook()
        self.jax = jax
        nc = _build_program(plan, niter)
        self.nc = nc

        in_names, out_names, out_avals = [], [], []
        partition_name = nc.partition_id_tensor.name if nc.partition_id_tensor else None
        for alloc in nc.m.functions[0].allocations:
            if not isinstance(alloc, _mb.MemoryLocationSet):
                continue
            name = alloc.memorylocations[0].name
            if alloc.kind == "ExternalInput":
                if name != partition_name:
                    in_names.append(name)
            elif alloc.kind == "ExternalOutput":
                out_names.append(name)
                out_avals.append(
                    jax.core.ShapedArray(
                        tuple(alloc.tensor_shape), _mb.dt.np(alloc.dtype)
                    )
                )

        self.in_names = in_names
        self.out_names = out_names
        self.out_avals = out_avals
        n_params = len(in_names)
        n_outs = len(out_avals)

        bind_in_names = list(in_names) + list(out_names)
        if partition_name is not None:
            bind_in_names.append(partition_name)

        def _body(*args):
            operands = list(args)
            if partition_name is not None:
                operands.append(partition_id_tensor())
            outs = _bass_exec_p.bind(
                *operands,
                out_avals=tuple(out_avals),
                in_names=tuple(bind_in_names),
                out_names=tuple(out_names),
                lowering_input_output_aliases=(),
                sim_require_finite=True,
                sim_require_nnan=True,
                nc=nc,
            )
            return tuple(outs)

        devices = jax.devices()[:NCORES]
        mesh = Mesh(np.asarray(devices), ("core",))
        self.sharding = NamedSharding(mesh, PartitionSpec("core"))
        in_specs = (PartitionSpec("core"),) * (n_params + n_outs)
        out_specs = (PartitionSpec("core"),) * n_outs
        donate = tuple(range(n_params, n_params + n_outs))
        self.fn = jax.jit(
            shard_map(
                _body,
                mesh=mesh,
                in_specs=in_specs,
                out_specs=out_specs,
                check_rep=False,
            ),
            donate_argnums=donate,
            keep_unused=True,
        )

    def place_inputs(self, in_maps):
        concat = [
            np.concatenate([np.asarray(m[name]) for m in in_maps], axis=0)
            for name in self.in_names
        ]
        return [self.jax.device_put(a, self.sharding) for a in concat]

    def make_zeros(self):
        return [
            self.jax.device_put(
                np.zeros((NCORES * av.shape[0], *av.shape[1:]), av.dtype),
                self.sharding,
            )
            for av in self.out_avals
        ]

    def run(self, dev_in, zeros):
        return self.fn(*dev_in, *zeros)


def _get_runner(plan, niter=1):
    key = (plan, niter)
    if key not in _RUNNERS:
        _RUNNERS[key] = _Runner(plan, niter)
    return _RUNNERS[key]


# -------------------------------------------------------------- public API --


def kernel(inputs, title_emb_mat, nbr_batch, nbr_job, nbr_title, nbr_mask):
    inputs = np.asarray(inputs, dtype=np.float32)
    emb = np.ascontiguousarray(inputs.reshape(B * P, D))
    prep = _sparse_host(
        inputs, title_emb_mat, nbr_batch, nbr_job, nbr_title, nbr_mask
    )
    out = np.zeros((B * P, 2 * D), dtype=np.float32)
    out[:, :D] = emb  # focal half of the concat is literally the input rows
    if prep is None:  # every slot masked: graph half is exactly zero
        return out
    plan, in_maps, row_focal_all, _ = prep

    runner = _get_runner(plan, 1)
    dev_in = runner.place_inputs(in_maps)
    outs = runner.run(dev_in, runner.make_zeros())
    outg_full = np.asarray(outs[runner.out_names.index("outg")]).astype(np.float32)
    total_rows = outg_full.shape[0] // NCORES
    for c in range(NCORES):
        outg = outg_full[c * total_rows : (c + 1) * total_rows]
        rf = row_focal_all[c]
        valid = rf >= 0
        out[c * R + rf[valid], D:] = outg[valid]
    return out


def bench(in_maps, plan, niters=(65, 257), reps=12):
    """Per-pass device time via on-device For_i iteration scaling; min-stat
    over reps cancels most of the axon RPC jitter."""
    results = {}
    for ni in niters:
        runner = _get_runner(plan, ni)
        dev_in = runner.place_inputs(in_maps)
        zeros = [runner.make_zeros() for _ in range(reps + 1)]
        out = runner.run(dev_in, zeros[0])
        for o in out:
            o.block_until_ready()
        ts = []
        for r in range(reps):
            t0 = time.perf_counter()
            outs = runner.run(dev_in, zeros[r + 1])
            for o in outs:
                o.block_until_ready()
            ts.append(time.perf_counter() - t0)
        results[ni] = min(ts)
        print(
            f"  niter={ni}: min {min(ts) * 1e3:.3f} ms  "
            f"(all: {', '.join(f'{t * 1e3:.2f}' for t in sorted(ts))})",
            flush=True,
        )
    ni_lo, ni_hi = min(niters), max(niters)
    per_pass = (results[ni_hi] - results[ni_lo]) / (ni_hi - ni_lo)
    return per_pass * 1e9, results

